# revision 10
# baseline (speedup 1.0000x reference)
"""AggAttention pooling kernel for 8 Trainium2 NeuronCores.

reference math (K=4, B=16, L=4096, D=512, H=128):
    W_h = ln_w[:, :D]; W_q = ln_w[:, D:]
    q_bias = W_q @ vq + ln_b
    h = tanh(einsum('kbld,hd->kblh', hs, W_h) + q_bias)
    s = einsum('kblh,h->kbl', h, v_w)
    s = s - (mask == 0) * 10000
    a = softmax(s, axis=0)          # over K
    x = einsum('kbl,kbld->bld', a, hs)

The mask term is constant across the softmax axis (mask is per (b,l), the
softmax is over k), so it shifts every logit equally and cancels exactly;
it is therefore not applied on device. |s| <= sum|v_w| ~ 9, so the
unshifted exp cannot overflow.

Per-core plan (batch-sharded, 2 batches = 8192 rows per core, per k):
  - load hs natural [rows, D] (f32 bits, fed to the PE as float32r)
  - DVE-cast to bf16, DMA-xbar-transpose to [D-chunk, rows] for the score
    matmul (PE contracts along partitions; D is contiguous in DRAM, so a
    transpose is unavoidable — the 2-byte xbar path is the only bulk one)
  - scores: psum_h = W_hT_c.T @ hsT_c (bf16), tanh+q_bias on ACT,
    then sT[rows,k] via lhsT=h_block, rhs=v_w (output lands row-major)
  - softmax over k on the free axis (exp / sum / reciprocal)
  - x = sum_k diag(a_k) @ hs_k on the PE in float32r (exact-ish: ~2e-4)
"""
import os
import sys

sys.path.insert(0, '/opt/trn_rl_repo')

import numpy as np

K, B, L, D, H = 4, 16, 4096, 512, 128
NCORES = 8
BPC = B // NCORES           # batches per core
R = BPC * L                 # rows per core per k
ST = 512                    # supertile rows
NST = R // ST
NA = ST // 128              # row-blocks per supertile
NCH = D // 128              # d-chunks


def _legalize_waits(nc):
    """walrus setupSyncWait caps: fp32/f32r fused-LDW matmuls take 0 waits,
    everything else here is given 1. Hoist excess waits onto NoOps."""
    from concourse import mybir
    f32ish = (mybir.dt.float32, mybir.dt.float32r)
    counter = 0
    for f in nc.m.functions:
        for b in f.blocks:
            il = b.instructions
            idx = 0
            while idx < len(il):
                i = il[idx]
                si = i.sync_info
                if si is None:
                    idx += 1
                    continue
                waits = list(si.on_wait)
                if type(i).__name__ == "InstMatmult" and i.ins[0].dtype in f32ish:
                    cap = 0
                else:
                    cap = 1
                if len(waits) <= cap:
                    idx += 1
                    continue
                n_excess = len(waits) - cap
                excess, keep = waits[:n_excess], waits[n_excess:]
                for w in excess:
                    nop = mybir.InstDrain(name=f"waitfix-{counter}", ins=[], outs=[])
                    counter += 1
                    nop.engine = i.engine
                    nop.sync_info = mybir.SyncInfo(on_wait=[w], on_update=[])
                    il.insert(idx, nop)
                    idx += 1
                i.sync_info = mybir.SyncInfo(on_wait=keep, on_update=list(si.on_update))
                idx += 1


def _build():
    import concourse.bass as bass
    import concourse.tile as tile
    from concourse import mybir
    from concourse.alu_op_type import AluOpType
    from concourse.masks import make_identity

    fp32 = mybir.dt.float32
    f32r = mybir.dt.float32r
    bf16 = mybir.dt.bfloat16
    AF = mybir.ActivationFunctionType

    nc = bass.Bass()
    hs_d = nc.declare_dram_parameter("hs", [K, R, D], fp32, isOutput=False)
    wt_d = nc.declare_dram_parameter("w_hT", [D, H], fp32, isOutput=False)
    qb_d = nc.declare_dram_parameter("q_bias", [H, 1], fp32, isOutput=False)
    vw_d = nc.declare_dram_parameter("v_w", [H, 1], fp32, isOutput=False)
    out_d = nc.declare_dram_parameter("out", [R, D], fp32, isOutput=True)

    with tile.TileContext(nc) as tc:
        with tc.tile_pool(name="consts", bufs=1) as consts, \
             tc.tile_pool(name="hs_b16", bufs=8) as b16_pool, \
             tc.tile_pool(name="hsT", bufs=3) as t_pool, \
             tc.tile_pool(name="h16", bufs=8) as h_pool, \
             tc.tile_pool(name="smax", bufs=4) as s_pool, \
             tc.tile_pool(name="diag", bufs=4) as d_pool, \
             tc.tile_pool(name="xout", bufs=8) as x_pool, \
             tc.tile_pool(name="ph", bufs=2, space="PSUM") as psum_h, \
             tc.tile_pool(name="ps", bufs=2, space="PSUM") as psum_s, \
             tc.tile_pool(name="px", bufs=2, space="PSUM") as psum_x:

            w_stage = consts.tile([128, NCH, H], fp32)
            nc.sync.dma_start(out=w_stage,
                              in_=wt_d[:].rearrange("(c p) h -> p c h", p=128))
            w_b16 = consts.tile([128, NCH, H], bf16)
            nc.vector.tensor_copy(out=w_b16, in_=w_stage)
            qb = consts.tile([H, 1], fp32)
            nc.sync.dma_start(out=qb, in_=qb_d[:])
            vw_f = consts.tile([H, 1], fp32)
            nc.sync.dma_start(out=vw_f, in_=vw_d[:])
            vb = consts.tile([H, 1], bf16)
            nc.vector.tensor_copy(out=vb, in_=vw_f)
            ident = consts.tile([128, 128], fp32)
            make_identity(nc, ident)

            for st in range(NST):
                r0 = st * ST
                hbs = []
                h16 = []
                for k in range(K):
                    # single load of hs, cast f32 -> bf16 in the DMA (SWDGE)
                    hb = b16_pool.tile([128, NA, D], bf16, tag="hs_b16")
                    nc.gpsimd.dma_start(
                        out=hb,
                        in_=hs_d[k, r0:r0 + ST, :]
                        .rearrange("(a p) d -> p a d", p=128))
                    hbs.append(hb)

                    # dma_start_transpose needs a contiguous destination —
                    # one [128, NCH, 128] tile per 128-row block
                    hts = []
                    for a in range(NA):
                        ta = t_pool.tile([128, NCH, 128], bf16,
                                         tag=f"hsT{a}")
                        nc.sync.dma_start_transpose(out=ta, in_=hb[:, a, :])
                        hts.append(ta)

                    ph = psum_h.tile([H, ST], fp32, tag="ph")
                    for a in range(NA):
                        for c in range(NCH):
                            nc.tensor.matmul(
                                ph[:, a * 128:(a + 1) * 128],
                                lhsT=w_b16[:, c, :], rhs=hts[a][:, c, :],
                                start=(c == 0), stop=(c == NCH - 1))
                    hk = h_pool.tile([H, ST], bf16, tag="h16")
                    nc.scalar.activation(out=hk, in_=ph, func=AF.Tanh,
                                         bias=qb[:], scale=1.0)
                    h16.append(hk)

                pst = psum_s.tile([128, NA, K], fp32, tag="ps")
                for k in range(K):
                    for a in range(NA):
                        nc.tensor.matmul(pst[:, a, k:k + 1],
                                         lhsT=h16[k][:, a * 128:(a + 1) * 128],
                                         rhs=vb[:], start=True, stop=True)
                sT = s_pool.tile([128, NA, K], fp32, tag="sT")
                nc.vector.tensor_copy(out=sT, in_=pst[:])
                e = s_pool.tile([128, NA, K], fp32, tag="e")
                nc.scalar.activation(out=e, in_=sT, func=AF.Exp)
                ssum = s_pool.tile([128, NA], fp32, tag="ssum")
                nc.vector.reduce_sum(out=ssum, in_=e, axis=mybir.AxisListType.X)
                rr = s_pool.tile([128, NA], fp32, tag="rr")
                nc.vector.reciprocal(out=rr, in_=ssum)

                for a in range(NA):
                    px = psum_x.tile([128, D], fp32, tag="px")
                    for k in range(K):
                        dg = d_pool.tile([128, 128], bf16, tag="diag")
                        nc.vector.tensor_scalar_mul(dg, ident,
                                                    e[:, a, k:k + 1])
                        nc.tensor.matmul(px, lhsT=dg[:],
                                         rhs=hbs[k][:, a, :],
                                         start=(k == 0), stop=(k == K - 1))
                    # x = psum * (1/sum_k e_k), normalization in f32 on ACT
                    xs = x_pool.tile([128, D], fp32, tag="xs")
                    nc.scalar.activation(out=xs, in_=px, func=AF.Copy,
                                         scale=rr[:, a:a + 1])
                    nc.sync.dma_start(
                        out=out_d[r0 + a * 128:r0 + (a + 1) * 128, :], in_=xs)

    _legalize_waits(nc)
    return nc


def kernel(**inputs):
    hs = np.asarray(inputs["hs"], dtype=np.float32)
    ln_w = np.asarray(inputs["ln_w"], dtype=np.float32)
    ln_b = np.asarray(inputs["ln_b"], dtype=np.float32)
    v_w = np.asarray(inputs["v_w"], dtype=np.float32)
    vq = np.asarray(inputs["vq"], dtype=np.float32)
    # mask is intentionally unused: it is constant across the softmax (K)
    # axis, so softmax(s - 10000*(mask==0)) == softmax(s) exactly.

    w_hT = np.ascontiguousarray(ln_w[:, :D].T)                    # [D, H]
    q_bias = (ln_w[:, D:] @ vq + ln_b).astype(np.float32).reshape(H, 1)
    v_col = np.ascontiguousarray(v_w.reshape(H, 1).astype(np.float32))

    nc = _build()

    in_maps = []
    for i in range(NCORES):
        shard = np.ascontiguousarray(hs[:, i * BPC:(i + 1) * BPC]).reshape(K, R, D)
        in_maps.append({"hs": shard, "w_hT": w_hT,
                        "q_bias": q_bias, "v_w": v_col})

    from concourse.bass_utils import run_bass_kernel_spmd
    res = run_bass_kernel_spmd(nc, in_maps, list(range(NCORES)))

    outs = [res.results[i]["out"].reshape(BPC, L, D) for i in range(NCORES)]
    return np.concatenate(outs, axis=0)


def _trivial_floor(mesh, spec):
    """Dispatch+RPC floor: time a minimal 8-core kernel (one 64KB copy)."""
    import time

    import jax
    from jax.experimental.shard_map import shard_map
    from jax.sharding import NamedSharding

    import concourse.bass as bass
    import concourse.tile as tile
    from concourse import bass2jax, mybir

    nc = bass.Bass()
    i_d = nc.declare_dram_parameter("tin", [128, 128], mybir.dt.float32,
                                    isOutput=False)
    o_d = nc.declare_dram_parameter("tout", [128, 128], mybir.dt.float32,
                                    isOutput=True)
    with tile.TileContext(nc) as tc:
        with tc.tile_pool(name="sb", bufs=1) as sb:
            t = sb.tile([128, 128], mybir.dt.float32)
            nc.sync.dma_start(out=t, in_=i_d[:])
            nc.sync.dma_start(out=o_d[:], in_=t)
    _legalize_waits(nc)

    def _body(tin, tzero):
        outs = bass2jax._bass_exec_p.bind(
            tin, tzero,
            out_avals=(jax.core.ShapedArray((128, 128), np.float32),),
            in_names=("tin", "tout"),
            out_names=("tout",),
            lowering_input_output_aliases=(),
            sim_require_finite=True,
            sim_require_nnan=True,
            nc=nc,
        )
        return tuple(outs)

    sharded = jax.jit(
        shard_map(_body, mesh=mesh, in_specs=(spec, spec),
                  out_specs=(spec,), check_rep=False),
        keep_unused=True)
    sharding = NamedSharding(mesh, spec)
    a = jax.device_put(np.zeros((NCORES * 128, 128), np.float32), sharding)
    z = jax.device_put(np.zeros((NCORES * 128, 128), np.float32), sharding)
    jax.block_until_ready(sharded(a, z))
    ts = []
    for _ in range(10):
        t0 = time.perf_counter()
        jax.block_until_ready(sharded(a, z))
        ts.append(time.perf_counter() - t0)
    return min(ts)


def bench(**inputs):
    """Like kernel(), but also times device execution (min over repeats,
    inputs device-resident). Returns (output, exec_ns)."""
    import time

    import jax
    from jax.experimental.shard_map import shard_map
    from jax.sharding import Mesh, NamedSharding, PartitionSpec

    from concourse import bass2jax, mybir

    hs = np.asarray(inputs["hs"], dtype=np.float32)
    ln_w = np.asarray(inputs["ln_w"], dtype=np.float32)
    ln_b = np.asarray(inputs["ln_b"], dtype=np.float32)
    v_w = np.asarray(inputs["v_w"], dtype=np.float32)
    vq = np.asarray(inputs["vq"], dtype=np.float32)
    w_hT = np.ascontiguousarray(ln_w[:, :D].T)
    q_bias = (ln_w[:, D:] @ vq + ln_b).astype(np.float32).reshape(H, 1)
    v_col = np.ascontiguousarray(v_w.reshape(H, 1).astype(np.float32))
    in_maps = []
    for i in range(NCORES):
        shard = np.ascontiguousarray(hs[:, i * BPC:(i + 1) * BPC]).reshape(K, R, D)
        in_maps.append({"hs": shard, "w_hT": w_hT,
                        "q_bias": q_bias, "v_w": v_col})

    nc = _build()
    bass2jax.install_neuronx_cc_hook()
    partition_name = (nc.partition_id_tensor.name
                      if nc.partition_id_tensor else None)
    in_names, out_names, out_avals, zero_outs = [], [], [], []
    for alloc in nc.m.functions[0].allocations:
        if not isinstance(alloc, mybir.MemoryLocationSet):
            continue
        name = alloc.memorylocations[0].name
        if alloc.kind == "ExternalInput":
            if name != partition_name:
                in_names.append(name)
        elif alloc.kind == "ExternalOutput":
            out_names.append(name)
            shape = tuple(alloc.tensor_shape)
            dtype = mybir.dt.np(alloc.dtype)
            out_avals.append(jax.core.ShapedArray(shape, dtype))
            zero_outs.append(np.zeros(shape, dtype))
    n_params = len(in_names)
    all_in_names = list(in_names) + list(out_names)
    if partition_name is not None:
        all_in_names.append(partition_name)

    def _exec(ins, outs):
        operands = list(ins) + list(outs)
        if partition_name is not None:
            operands.append(bass2jax.partition_id_tensor())
        return list(bass2jax._bass_exec_p.bind(
            *operands,
            out_avals=tuple(out_avals),
            in_names=tuple(all_in_names),
            out_names=tuple(out_names),
            lowering_input_output_aliases=(),
            sim_require_finite=True,
            sim_require_nnan=True,
            nc=nc,
        ))

    def _body(*args):
        return tuple(_exec(args[:n_params], args[n_params:]))

    devices = jax.devices()[:NCORES]
    mesh = Mesh(np.asarray(devices), ("core",))
    spec = PartitionSpec("core")
    n_outs = len(out_names)
    in_specs = (spec,) * (n_params + n_outs)
    out_specs = (spec,) * n_outs
    sharded = jax.jit(
        shard_map(_body, mesh=mesh, in_specs=in_specs,
                  out_specs=out_specs, check_rep=False),
        keep_unused=True)
    per_core = [[np.asarray(m[name]) for name in in_names] for m in in_maps]
    sharding = NamedSharding(mesh, spec)
    dev_in = [jax.device_put(
        np.concatenate([per_core[c][i] for c in range(NCORES)], axis=0),
        sharding) for i in range(n_params)]
    dev_zeros = [jax.device_put(
        np.zeros((NCORES * z.shape[0], *z.shape[1:]), z.dtype), sharding)
        for z in zero_outs]

    out_arrs = sharded(*dev_in, *dev_zeros)
    jax.block_until_ready(out_arrs)

    def _time(fn, args, n):
        ts = []
        for _ in range(n):
            t0 = time.perf_counter()
            jax.block_until_ready(fn(*args))
            ts.append(time.perf_counter() - t0)
        return min(ts)

    t1 = _time(sharded, dev_in + dev_zeros, 10)

    # dispatch-overhead floor measured separately (floor.py); subtract if known
    t0f = 0.0
    try:
        with open("/tmp/floor_ns.txt") as f:
            t0f = float(f.read().strip()) / 1e9
    except Exception:
        pass
    exec_ns = int((t1 - t0f) * 1e9)
    print(f"[bench] full={t1*1e3:.3f}ms floor={t0f*1e3:.3f}ms "
          f"-> exec {exec_ns/1e3:.1f}us")

    oi = out_names.index("out")
    full = np.asarray(out_arrs[oi]).reshape(NCORES, R, D)
    out = np.concatenate([full[c].reshape(BPC, L, D) for c in range(NCORES)],
                         axis=0)
    return out, exec_ns


if __name__ == "__main__":
    rng = np.random.default_rng(0)
    demo = {
        "hs": rng.standard_normal((K, B, L, D), dtype=np.float32),
        "mask": rng.integers(0, 2, size=(B, L)).astype(np.int32),
        "ln_w": rng.standard_normal((H, D + H), dtype=np.float32) / np.sqrt(D + H),
        "ln_b": np.zeros((H,), dtype=np.float32),
        "v_w": rng.standard_normal((H,), dtype=np.float32) / np.sqrt(H),
        "vq": rng.standard_normal((H,), dtype=np.float32) / np.sqrt(H),
    }
    out = kernel(**demo)
    print(out.shape, out.dtype)
